# revision 24
# baseline (speedup 1.0000x reference)
"""Trainium2 distributed kernel for AnatomicalConsistencyLoss (v2).

Sharding: 8 cores = (batch b in {0,1}) x (depth quarter q in {0..3});
each core owns 40 output D-planes (full H,W) of one batch element.

Per-core layout: partitions p = hb*42 + dl (3 h-blocks x 42 d-planes
incl 1-plane halo), free axis = (h_local 56 incl halo, w 164 padded)
bf16.  The Sobel separable conv is split across engines:
  - W passes (stride-1 axis): VectorE shifted adds at DVE 2x bf16 mode,
    with the odd-offset center tap (2*x<<1) on ScalarE.
  - H passes: VectorE shifted adds at even 164-elem offsets (2x mode).
  - D passes: TensorE matmuls with banded [126,126] conv matrices
    (S=[1,2,1], D=[-1,0,1] per h-block, zero columns at d-halo
    outputs), streaming 3-h-row chunks into PSUM fp32.
Squares run on ScalarE straight out of PSUM (fused valid-region
compaction + accum_out partial sums); dot products are DVE muls from
PSUM; sqrt on ScalarE (+accum for the mag cross term); 1/sqrt via the
custom-DVE fast reciprocal; the cosine sum via tensor_tensor_reduce.

Per-core output: [128, 160] fp32 accum slots (8 per h-band x 18 bands:
3x sum gp^2, 3x sum gt^2, sum sqrt(q), sum dot/sqrt(q)); host reduces.
"""

import sys

import numpy as np

sys.path.insert(0, "/opt/trn_rl_repo")

import ml_dtypes

N_CORES = 8
DL = 42            # d planes incl halo
HB = 3             # h blocks
HL = 56            # h_local rows incl halo
WR = 164           # padded w row (4B-aligned rows)
NP_ = 126          # used partitions
FREE = HL * WR     # 9184
NBAND = 9          # 54 valid h rows / 6
BH = 6             # h rows per band
BF = 3 * WR        # 492 cols per matmul chunk (<= 512 fp32 bank)
PF = 1024          # PSUM tile cols (2 banks; rows 0-2 at 0, 3-5 at 512)
CF = BH * 160      # 960 compact cols
NVOX = 2 * 160 * 160 * 160
WEIGHT = 0.2

_cache = {}


def _build_M():
    MS = np.zeros((128, 252), np.float32)
    for hb in range(HB):
        for do in range(40):
            j = hb * DL + do
            MS[hb * DL + do, j] += 1.0
            MS[hb * DL + do + 1, j] += 2.0
            MS[hb * DL + do + 2, j] += 1.0
            MS[hb * DL + do, 126 + j] += -1.0
            MS[hb * DL + do + 2, 126 + j] += 1.0
    return MS


def _build():
    import concourse.bacc as bacc
    import concourse.tile as tile
    from concourse import mybir

    f32 = mybir.dt.float32
    bf16 = mybir.dt.bfloat16
    AF = mybir.ActivationFunctionType
    ALU = mybir.AluOpType

    nc = bacc.Bacc(
        "TRN2",
        target_bir_lowering=False,
        debug=False,
        enable_asserts=False,
        num_devices=N_CORES,
    )
    xp_d = nc.dram_tensor("pred", [128, FREE], bf16, kind="ExternalInput")
    xt_d = nc.dram_tensor("targ", [128, FREE], bf16, kind="ExternalInput")
    mm_d = nc.dram_tensor("mconst", [128, 252], bf16, kind="ExternalInput")
    out_d = nc.dram_tensor("out", [128, 160], f32, kind="ExternalOutput")
    jnk_d = nc.dram_tensor("jnk", [128, 2560], bf16, kind="ExternalOutput")

    QN = FREE // 4  # 2296, multiple of WR

    with tile.TileContext(nc) as tc:
        with tc.tile_pool(name="pers", bufs=1) as pers, \
             tc.tile_pool(name="conv", bufs=1) as conv, \
             tc.tile_pool(name="band", bufs=2) as band, \
             tc.psum_pool(name="ps", bufs=1) as ps:
            accs = pers.tile([128, 160], f32, tag="accs")
            msd = pers.tile([128, 252], bf16, tag="msd")
            nc.sync.dma_start(out=msd[:, :], in_=mm_d[:, :])
            bias_t = pers.tile([128, 1], f32, tag="bias")
            nc.vector.memset(bias_t[:, :], 1e-30)

            xs = {}
            for name, dram in (("p", xp_d), ("t", xt_d)):
                x = conv.tile([128, FREE], bf16, tag=f"x_{name}")
                for qt in range(4):
                    a = qt * QN
                    nc.sync.dma_start(out=x[:, a:a + QN], in_=dram[:, a:a + QN])
                dw = conv.tile([128, FREE], bf16, tag=f"dw_{name}")
                so = conv.tile([128, FREE], bf16, tag=f"so_{name}")
                for qt in range(4):
                    a = qt * QN
                    n = QN if qt < 3 else QN - 2
                    u = conv.tile([128, QN], bf16, tag="u")
                    xd = conv.tile([128, QN], bf16, tag="xd")
                    nc.vector.tensor_sub(dw[:NP_, a:a + n],
                                         x[:NP_, a + 2:a + n + 2],
                                         x[:NP_, a:a + n])
                    nc.vector.tensor_add(u[:NP_, 0:n],
                                         x[:NP_, a:a + n],
                                         x[:NP_, a + 2:a + n + 2])
                    nc.scalar.activation(xd[:NP_, 0:n],
                                         x[:NP_, a + 1:a + n + 1],
                                         AF.Identity, scale=2.0)
                    nc.vector.tensor_add(so[:NP_, a:a + n],
                                         u[:NP_, 0:n],
                                         xd[:NP_, 0:n])
                xs[name] = (dw, so)

            def vps(t):
                """Valid [126, 2, 3, 160] view of a [128, PF] PSUM tile.

                Row r (0..5) lives at col 512*(r//3) + 164*(r%3) so each
                3-row chunk stays inside one 512-fp32 PSUM bank.
                """
                return (t[0:NP_, :]
                        .rearrange("p (c q) -> p c q", c=2)[:, :, 0:BF]
                        .rearrange("p c (k w) -> p c k w", k=3)[:, :, :, 0:160])

            def vcm(t):
                """[126, 2, 3, 160] view of a [128, CF] compact tile."""
                return t[0:NP_, :].rearrange("p (c k w) -> p c k w", c=2, k=3)


            for bi in range(NBAND):
                a = BH * bi * WR
                gtc = []
                grads_p = []
                gps = [ps.tile([128, PF], f32, tag=f"g{ci}", name=f"g{ci}")
                       for ci in range(3)]
                for name in ("t", "p"):
                    dw, so = xs[name]
                    uh = band.tile([128, 7 * WR], bf16, tag="uh")
                    hx = band.tile([128, BH * WR], bf16, tag="hx")
                    nc.vector.tensor_add(uh[:NP_, :],
                                         dw[:NP_, a:a + 7 * WR],
                                         dw[:NP_, a + WR:a + 8 * WR])
                    nc.vector.tensor_add(hx[:NP_, :],
                                         uh[:NP_, 0:BH * WR],
                                         uh[:NP_, WR:WR + BH * WR])
                    hy = band.tile([128, BH * WR], bf16, tag="hy")
                    nc.vector.tensor_sub(hy[:NP_, :],
                                         so[:NP_, a + 2 * WR:a + (2 + BH) * WR],
                                         so[:NP_, a:a + BH * WR])
                    uh2 = band.tile([128, 7 * WR], bf16, tag="uh")
                    hz = band.tile([128, BH * WR], bf16, tag="hz")
                    nc.vector.tensor_add(uh2[:NP_, :],
                                         so[:NP_, a:a + 7 * WR],
                                         so[:NP_, a + WR:a + 8 * WR])
                    nc.vector.tensor_add(hz[:NP_, :],
                                         uh2[:NP_, 0:BH * WR],
                                         uh2[:NP_, WR:WR + BH * WR])
                    for ci, (rhs, wcol) in enumerate(
                            ((hx, 0), (hy, 0), (hz, 126))):
                        for ch in range(2):
                            nc.tensor.matmul(
                                out=gps[ci][0:NP_, 512 * ch:512 * ch + BF],
                                lhsT=msd[0:NP_, wcol:wcol + 126],
                                rhs=rhs[0:NP_, BF * ch:BF * (ch + 1)],
                                start=True, stop=True)
                    if name == "t":
                        # evacuate t-gradients so p can reuse the banks
                        for ci in range(3):
                            g = band.tile([128, CF], bf16, tag=f"gtc{ci}")
                            nc.scalar.activation(vcm(g), vps(gps[ci]),
                                                 AF.Identity)
                            gtc.append(g)
                    else:
                        grads_p = gps

                # squares (ScalarE, fused accums)
                sqs = {"p": [], "t": []}
                for ni, (name, srcs) in enumerate(
                        (("p", [vps(g) for g in grads_p]),
                         ("t", [vcm(g) for g in gtc]))):
                    for ci, gv in enumerate(srcs):
                        sq = band.tile([128, CF], bf16, tag=f"sq{name}{ci}")
                        sv = vcm(sq)
                        k = 3 * ni + ci
                        nc.scalar.activation(
                            sv, gv, AF.Square,
                            accum_out=accs[0:NP_, 8 * bi + k:8 * bi + k + 1])
                        if bi == NBAND - 1:
                            nc.sync.dma_start(
                                out=jnk_d[:, 320 * k:320 * (k + 1)],
                                in_=sq[:, 640:960])
                        sqs[name].append(sq)

                s_p = band.tile([128, CF], bf16, tag="s_p")
                s_t = band.tile([128, CF], bf16, tag="s_t")
                dot = band.tile([128, CF], bf16, tag="dot")
                t0 = band.tile([128, CF], bf16, tag="t0")
                nc.vector.tensor_add(t0[:NP_, :], sqs["p"][0][:NP_, :],
                                     sqs["p"][1][:NP_, :])
                nc.vector.tensor_add(s_p[:NP_, :], t0[:NP_, :],
                                     sqs["p"][2][:NP_, :])
                nc.vector.tensor_add(t0[:NP_, :], sqs["t"][0][:NP_, :],
                                     sqs["t"][1][:NP_, :])
                nc.vector.tensor_add(s_t[:NP_, :], t0[:NP_, :],
                                     sqs["t"][2][:NP_, :])

                # dot products (DVE, single PSUM operand); reuse sqp memory
                ms = []
                for ci in range(3):
                    m = band.tile([128, CF], bf16, tag=f"sqp{ci}", name=f"m{ci}")
                    nc.vector.tensor_mul(vcm(m), vps(grads_p[ci]),
                                         vcm(gtc[ci]))
                    ms.append(m)
                nc.vector.tensor_add(t0[:NP_, :], ms[0][:NP_, :],
                                     ms[1][:NP_, :])
                nc.vector.tensor_add(dot[:NP_, :], t0[:NP_, :],
                                     ms[2][:NP_, :])
                q = band.tile([128, CF], bf16, tag="q")
                nc.vector.tensor_mul(q[:NP_, :], s_p[:NP_, :], s_t[:NP_, :])

                sqq = band.tile([128, CF], f32, tag="sqq")
                nc.scalar.activation(
                    vcm(sqq), vcm(q), AF.Sqrt, bias=bias_t[0:NP_, 0:1],
                    accum_out=accs[0:NP_, 8 * bi + 6:8 * bi + 7])
                if bi == NBAND - 1:
                    nc.sync.dma_start(out=jnk_d[:, 1920:2240],
                                      in_=q[:, 640:960])
                r = band.tile([128, CF], f32, tag="r")
                nc.vector.reciprocal_approx_fast(out=r[:NP_, :],
                                                 in_=sqq[:NP_, :])
                cj = band.tile([128, CF], bf16, tag="cj")
                nc.vector.tensor_mul(cj[:NP_, :], dot[:NP_, :], r[:NP_, :])
                nc.scalar.activation(
                    vcm(t0), vcm(cj), AF.Identity,
                    accum_out=accs[0:NP_, 8 * bi + 7:8 * bi + 8])
                if bi == NBAND - 1:
                    nc.sync.dma_start(out=jnk_d[:, 2240:2560],
                                      in_=cj[:, 640:960])

            nc.sync.dma_start(out=out_d[:, :], in_=accs[:, :])

    nc.compile()
    return nc


def _shard_inputs(pred, target):
    bf = ml_dtypes.bfloat16
    in_maps = []
    padded = {}
    for name, x in (("pred", pred), ("targ", target)):
        per_b = []
        for b in range(2):
            G = np.zeros((164, 164, 164), np.float32)
            G[1:161, 1:161, 1:161] = x[b, 0]
            per_b.append(G)
        padded[name] = per_b

    for core in range(N_CORES):
        b, q = divmod(core, 4)
        m = {}
        for name in ("pred", "targ"):
            G = padded[name][b]
            slab = G[40 * q:40 * q + DL]          # [42, 164, 164]
            blocks = np.stack([slab[:, hb * 54:hb * 54 + HL, :]
                               for hb in range(HB)])  # [3, 42, 56, 164]
            arr = np.zeros((128, FREE), bf)
            arr[:NP_] = blocks.reshape(NP_, FREE).astype(bf)
            m[name] = arr
        m["mconst"] = _build_M().astype(bf)
        in_maps.append(m)
    return in_maps


def run(pred, target, trace=False):
    from concourse.bass_utils import run_bass_kernel_spmd

    pred = np.asarray(pred, dtype=np.float32)
    target = np.asarray(target, dtype=np.float32)
    assert pred.shape == (2, 1, 160, 160, 160)

    if "nc" not in _cache:
        _cache["nc"] = _build()
    nc = _cache["nc"]

    in_maps = _shard_inputs(pred, target)
    res = None
    for attempt in range(3):
        try:
            res = run_bass_kernel_spmd(
                nc, in_maps, core_ids=list(range(N_CORES)), trace=trace)
            break
        except Exception:
            if attempt == 2:
                raise
            import time as _time
            _time.sleep(5)

    sp = st = sq = cs = 0.0
    nb8 = 8 * NBAND
    for core_out in res.results:
        o = np.asarray(core_out["out"], np.float64)
        sl = o[:NP_, :nb8].reshape(NP_, NBAND, 8)
        tk = sl.sum(axis=(0, 1))                       # incl band-8 junk rows
        # subtract hb2 junk (h>=160): last band rows 4:5, partitions 84:126
        j = np.asarray(core_out["jnk"], np.float64)[2 * DL:NP_, :]
        for k in range(6):
            tk[k] -= j[:, 320 * k:320 * (k + 1)].sum()
        tk[6] -= np.sqrt(j[:, 1920:2240] + 1e-30).sum()
        tk[7] -= j[:, 2240:2560].sum()
        sp += tk[0:3].sum()
        st += tk[3:6].sum()
        sq += tk[6]
        cs += tk[7]

    mag = sp + st - 2.0 * sq
    loss = WEIGHT * (mag / NVOX + 1.0 - cs / NVOX)
    return np.float32(loss), res.exec_time_ns


def kernel(pred, target):
    loss, _ = run(pred, target, trace=False)
    return loss


# revision 26
# speedup vs baseline: 1.0768x; 1.0768x over previous
"""Trainium2 distributed kernel for AnatomicalConsistencyLoss (v2).

Sharding: 8 cores = (batch b in {0,1}) x (depth quarter q in {0..3});
each core owns 40 output D-planes (full H,W) of one batch element.

Per-core layout: partitions p = hb*42 + dl (3 h-blocks x 42 d-planes
incl 1-plane halo), free axis = (h_local 56 incl halo, w 164 padded)
bf16.  The Sobel separable conv is split across engines:
  - W passes (stride-1 axis): VectorE shifted adds at DVE 2x bf16 mode,
    with the odd-offset center tap (2*x<<1) on ScalarE.
  - H passes: VectorE shifted adds at even 164-elem offsets (2x mode).
  - D passes: TensorE matmuls with banded [126,126] conv matrices
    (S=[1,2,1], D=[-1,0,1] per h-block, zero columns at d-halo
    outputs), streaming 3-h-row chunks into PSUM fp32.
Squares run on ScalarE straight out of PSUM (fused valid-region
compaction + accum_out partial sums); dot products are DVE muls from
PSUM; sqrt on ScalarE (+accum for the mag cross term); 1/sqrt via the
custom-DVE fast reciprocal; the cosine sum via tensor_tensor_reduce.

Per-core output: [128, 160] fp32 accum slots (8 per h-band x 18 bands:
3x sum gp^2, 3x sum gt^2, sum sqrt(q), sum dot/sqrt(q)); host reduces.
"""

import sys

import numpy as np

sys.path.insert(0, "/opt/trn_rl_repo")

import ml_dtypes

N_CORES = 8
DL = 42            # d planes incl halo
HB = 3             # h blocks
HL = 56            # h_local rows incl halo
WR = 164           # padded w row (4B-aligned rows)
NP_ = 126          # used partitions
FREE = HL * WR     # 9184
NBAND = 9          # 54 valid h rows / 6
BH = 6             # h rows per band
BF = 3 * WR        # 492 cols per matmul chunk (<= 512 fp32 bank)
PF = 1024          # PSUM tile cols (2 banks; rows 0-2 at 0, 3-5 at 512)
CF = BH * 160      # 960 compact cols
NVOX = 2 * 160 * 160 * 160
WEIGHT = 0.2

_cache = {}


def _build_M():
    MS = np.zeros((128, 252), np.float32)
    for hb in range(HB):
        for do in range(40):
            j = hb * DL + do
            MS[hb * DL + do, j] += 1.0
            MS[hb * DL + do + 1, j] += 2.0
            MS[hb * DL + do + 2, j] += 1.0
            MS[hb * DL + do, 126 + j] += -1.0
            MS[hb * DL + do + 2, 126 + j] += 1.0
    return MS


def _build():
    import concourse.bacc as bacc
    import concourse.tile as tile
    from concourse import mybir

    f32 = mybir.dt.float32
    bf16 = mybir.dt.bfloat16
    AF = mybir.ActivationFunctionType
    ALU = mybir.AluOpType

    nc = bacc.Bacc(
        "TRN2",
        target_bir_lowering=False,
        debug=False,
        enable_asserts=False,
        num_devices=N_CORES,
    )
    xp_d = nc.dram_tensor("pred", [128, FREE], bf16, kind="ExternalInput")
    xt_d = nc.dram_tensor("targ", [128, FREE], bf16, kind="ExternalInput")
    mm_d = nc.dram_tensor("mconst", [128, 252], bf16, kind="ExternalInput")
    out_d = nc.dram_tensor("out", [128, 160], f32, kind="ExternalOutput")
    jnk_d = nc.dram_tensor("jnk", [128, 2560], bf16, kind="ExternalOutput")

    QN = FREE // 4  # 2296, multiple of WR

    with tile.TileContext(nc) as tc:
        with tc.tile_pool(name="pers", bufs=1) as pers, \
             tc.tile_pool(name="conv", bufs=1) as conv, \
             tc.tile_pool(name="band", bufs=2) as band, \
             tc.psum_pool(name="ps", bufs=1) as ps:
            accs = pers.tile([128, 160], f32, tag="accs")
            msd = pers.tile([128, 252], bf16, tag="msd")
            nc.sync.dma_start(out=msd[:, :], in_=mm_d[:, :])
            bias_t = pers.tile([128, 1], f32, tag="bias")
            nc.vector.memset(bias_t[:, :], 1e-30)

            HF = 54 * WR   # full-tile h-conv output size
            hs = {}
            for name, dram in (("p", xp_d), ("t", xt_d)):
                x = conv.tile([128, FREE], bf16, tag="x")
                for qt in range(4):
                    a = qt * QN
                    nc.sync.dma_start(out=x[:, a:a + QN], in_=dram[:, a:a + QN])
                dw = conv.tile([128, FREE], bf16, tag=f"dw_{name}")
                so = conv.tile([128, FREE], bf16, tag=f"so_{name}")
                for qt in range(4):
                    a = qt * QN
                    n = QN if qt < 3 else QN - 2
                    u = conv.tile([128, QN], bf16, tag=f"hx_{name}",
                                  name="u")
                    xd = conv.tile([128, QN], bf16, tag="xd", name="xd")
                    nc.vector.tensor_sub(dw[:NP_, a:a + n],
                                         x[:NP_, a + 2:a + n + 2],
                                         x[:NP_, a:a + n])
                    nc.vector.tensor_add(u[:NP_, 0:n],
                                         x[:NP_, a:a + n],
                                         x[:NP_, a + 2:a + n + 2])
                    nc.scalar.activation(xd[:NP_, 0:n],
                                         x[:NP_, a + 1:a + n + 1],
                                         AF.Identity, scale=2.0)
                    nc.vector.tensor_add(so[:NP_, a:a + n],
                                         u[:NP_, 0:n],
                                         xd[:NP_, 0:n])
                # full-tile h-convs (x buffer is dead, reuse it for uh)
                uhf = conv.tile([128, 55 * WR], bf16, tag="x",
                                name="uhf")
                nc.vector.tensor_add(uhf[:NP_, :],
                                     dw[:NP_, 0:55 * WR],
                                     dw[:NP_, WR:56 * WR])
                hx = conv.tile([128, HF], bf16, tag=f"hx_{name}", name="hx")
                nc.vector.tensor_add(hx[:NP_, :],
                                     uhf[:NP_, 0:HF],
                                     uhf[:NP_, WR:WR + HF])
                hy = conv.tile([128, HF], bf16, tag=f"dw_{name}", name="hy")
                nc.vector.tensor_sub(hy[:NP_, :],
                                     so[:NP_, 2 * WR:2 * WR + HF],
                                     so[:NP_, 0:HF])
                uh2 = conv.tile([128, 55 * WR], bf16, tag="x",
                                name="uh2")
                nc.vector.tensor_add(uh2[:NP_, :],
                                     so[:NP_, 0:55 * WR],
                                     so[:NP_, WR:56 * WR])
                hz = conv.tile([128, HF], bf16, tag=f"so_{name}", name="hz")
                nc.vector.tensor_add(hz[:NP_, :],
                                     uh2[:NP_, 0:HF],
                                     uh2[:NP_, WR:WR + HF])
                hs[name] = (hx, hy, hz)

            def vps(t):
                """Valid [126, 2, 3, 160] view of a [128, PF] PSUM tile.

                Row r (0..5) lives at col 512*(r//3) + 164*(r%3) so each
                3-row chunk stays inside one 512-fp32 PSUM bank.
                """
                return (t[0:NP_, :]
                        .rearrange("p (c q) -> p c q", c=2)[:, :, 0:BF]
                        .rearrange("p c (k w) -> p c k w", k=3)[:, :, :, 0:160])

            def vcm(t):
                """[126, 2, 3, 160] view of a [128, CF] compact tile."""
                return t[0:NP_, :].rearrange("p (c k w) -> p c k w", c=2, k=3)


            for bi in range(NBAND):
                a = BH * bi * WR
                gtc = None
                grads_p = []
                gps = [ps.tile([128, PF], f32, tag=f"g{ci}", name=f"g{ci}")
                       for ci in range(3)]
                for name in ("t", "p"):
                    for ci, rhs in enumerate(hs[name]):
                        wcol = 126 if ci == 2 else 0
                        for ch in range(2):
                            b0 = a + BF * ch
                            nc.tensor.matmul(
                                out=gps[ci][0:NP_, 512 * ch:512 * ch + BF],
                                lhsT=msd[0:NP_, wcol:wcol + 126],
                                rhs=rhs[0:NP_, b0:b0 + BF],
                                start=True, stop=True)
                    if name == "t":
                        # evacuate t-gradients so p can reuse the banks
                        gtc = band.tile([128, 3 * CF], bf16, tag="gtc")
                        for ci in range(3):
                            nc.scalar.activation(
                                vcm(gtc[:, ci * CF:(ci + 1) * CF]),
                                vps(gps[ci]), AF.Identity)
                    else:
                        grads_p = gps

                # squares (ScalarE, fused accums); t side in one op
                sqs = []
                for ci in range(3):
                    sq = band.tile([128, CF], bf16, tag=f"sqp{ci}",
                                   name=f"sqp{ci}")
                    nc.scalar.activation(
                        vcm(sq), vps(grads_p[ci]), AF.Square,
                        accum_out=accs[0:NP_, 8 * bi + ci:8 * bi + ci + 1])
                    if bi == NBAND - 1:
                        nc.sync.dma_start(
                            out=jnk_d[:, 320 * ci:320 * (ci + 1)],
                            in_=sq[:, 640:960])
                    sqs.append(sq)
                sqta = band.tile([128, 3 * CF], bf16, tag="sqta")
                nc.scalar.activation(
                    sqta[0:NP_, :], gtc[0:NP_, :], AF.Square,
                    accum_out=accs[0:NP_, 8 * bi + 3:8 * bi + 4])
                if bi == NBAND - 1:
                    for ci in range(3):
                        nc.sync.dma_start(
                            out=jnk_d[:, 320 * (3 + ci):320 * (4 + ci)],
                            in_=sqta[:, ci * CF + 640:ci * CF + 960])

                s_p = band.tile([128, CF], bf16, tag="s_p")
                s_t = band.tile([128, CF], bf16, tag="s_t")
                dot = pers.tile([128, CF], bf16, tag="dot")
                t0 = band.tile([128, CF], bf16, tag="t0")
                nc.vector.tensor_add(t0[:NP_, :], sqs[0][:NP_, :],
                                     sqs[1][:NP_, :])
                nc.vector.tensor_add(s_p[:NP_, :], t0[:NP_, :],
                                     sqs[2][:NP_, :])
                nc.vector.tensor_add(t0[:NP_, :], sqta[:NP_, 0:CF],
                                     sqta[:NP_, CF:2 * CF])
                nc.vector.tensor_add(s_t[:NP_, :], t0[:NP_, :],
                                     sqta[:NP_, 2 * CF:3 * CF])

                # dot products (DVE, single PSUM operand); reuse sqp memory
                ms = []
                for ci in range(3):
                    m = band.tile([128, CF], bf16, tag=f"sqp{ci}",
                                  name=f"m{ci}")
                    nc.vector.tensor_mul(vcm(m), vps(grads_p[ci]),
                                         vcm(gtc[:, ci * CF:(ci + 1) * CF]))
                    ms.append(m)
                nc.vector.tensor_add(t0[:NP_, :], ms[0][:NP_, :],
                                     ms[1][:NP_, :])
                nc.vector.tensor_add(dot[:NP_, :], t0[:NP_, :],
                                     ms[2][:NP_, :])
                q = band.tile([128, CF], bf16, tag="q")
                nc.vector.tensor_mul(q[:NP_, :], s_p[:NP_, :], s_t[:NP_, :])

                sqq = band.tile([128, CF], f32, tag="sqq")
                nc.scalar.activation(
                    vcm(sqq), vcm(q), AF.Sqrt, bias=bias_t[0:NP_, 0:1],
                    accum_out=accs[0:NP_, 8 * bi + 6:8 * bi + 7])
                if bi == NBAND - 1:
                    nc.sync.dma_start(out=jnk_d[:, 1920:2240],
                                      in_=q[:, 640:960])
                r = pers.tile([128, CF], f32, tag="r")
                nc.vector.reciprocal_approx_fast(out=r[:NP_, :],
                                                 in_=sqq[:NP_, :])
                cj = band.tile([128, CF], bf16, tag="cj")
                nc.vector.tensor_mul(cj[:NP_, :], dot[:NP_, :], r[:NP_, :])
                cjunk = pers.tile([128, CF], bf16, tag="cjunk")
                nc.scalar.activation(
                    vcm(cjunk), vcm(cj), AF.Identity,
                    accum_out=accs[0:NP_, 8 * bi + 7:8 * bi + 8])
                if bi == NBAND - 1:
                    nc.sync.dma_start(out=jnk_d[:, 2240:2560],
                                      in_=cj[:, 640:960])

            nc.sync.dma_start(out=out_d[:, :], in_=accs[:, :])

    nc.compile()
    return nc


def _shard_inputs(pred, target):
    bf = ml_dtypes.bfloat16
    in_maps = []
    padded = {}
    for name, x in (("pred", pred), ("targ", target)):
        per_b = []
        for b in range(2):
            G = np.zeros((164, 164, 164), np.float32)
            G[1:161, 1:161, 1:161] = x[b, 0]
            per_b.append(G)
        padded[name] = per_b

    for core in range(N_CORES):
        b, q = divmod(core, 4)
        m = {}
        for name in ("pred", "targ"):
            G = padded[name][b]
            slab = G[40 * q:40 * q + DL]          # [42, 164, 164]
            blocks = np.stack([slab[:, hb * 54:hb * 54 + HL, :]
                               for hb in range(HB)])  # [3, 42, 56, 164]
            arr = np.zeros((128, FREE), bf)
            arr[:NP_] = blocks.reshape(NP_, FREE).astype(bf)
            m[name] = arr
        m["mconst"] = _build_M().astype(bf)
        in_maps.append(m)
    return in_maps


def run(pred, target, trace=False):
    from concourse.bass_utils import run_bass_kernel_spmd

    pred = np.asarray(pred, dtype=np.float32)
    target = np.asarray(target, dtype=np.float32)
    assert pred.shape == (2, 1, 160, 160, 160)

    if "nc" not in _cache:
        _cache["nc"] = _build()
    nc = _cache["nc"]

    in_maps = _shard_inputs(pred, target)
    res = None
    for attempt in range(3):
        try:
            res = run_bass_kernel_spmd(
                nc, in_maps, core_ids=list(range(N_CORES)), trace=trace)
            break
        except Exception:
            if attempt == 2:
                raise
            import time as _time
            _time.sleep(5)

    sp = st = sq = cs = 0.0
    nb8 = 8 * NBAND
    for core_out in res.results:
        o = np.asarray(core_out["out"], np.float64)
        sl = o[:NP_, :nb8].reshape(NP_, NBAND, 8)
        tk = sl.sum(axis=(0, 1))                       # incl band-8 junk rows
        # subtract hb2 junk (h>=160): last band rows 4:5, partitions 84:126
        j = np.asarray(core_out["jnk"], np.float64)[2 * DL:NP_, :]
        for k in range(3):
            tk[k] -= j[:, 320 * k:320 * (k + 1)].sum()
        tk[3] -= j[:, 960:1920].sum()
        tk[6] -= np.sqrt(j[:, 1920:2240] + 1e-30).sum()
        tk[7] -= j[:, 2240:2560].sum()
        sp += tk[0:3].sum()
        st += tk[3]
        sq += tk[6]
        cs += tk[7]

    mag = sp + st - 2.0 * sq
    loss = WEIGHT * (mag / NVOX + 1.0 - cs / NVOX)
    return np.float32(loss), res.exec_time_ns


def kernel(pred, target):
    loss, _ = run(pred, target, trace=False)
    return loss
